# revision 31
# baseline (speedup 1.0000x reference)
"""Multi-head attention (B=2, S=2048, H=1024, 16 heads x 64) on 8 Trainium2 cores.

Sharding: tensor-parallel over heads x data-parallel over batch.
Core c handles batch b = c//4 and heads [4*(c%4), 4*(c%4)+4).

Per-core kernel (bf16 matmul operands, fp32 PSUM accumulation):
  - inputs arrive host-transposed (contraction dim outermost) as part of the
    sharding step; SWDGE cast-loads move them straight into bf16 SBUF
  - QKV projection produces qT/kT ([headdim, S], head pairs stacked on
    partitions) and V in natural [S, headdim] layout augmented with ones
    columns; the ones rows of the PV output give the softmax denominators
    replicated across 64 partitions, so normalization needs no broadcast
  - scores are computed transposed (sT = kT_chunk.T @ qT block) so the
    softmax k-reduction lands on the partition axis and probs come out in
    the [k, q] layout PV needs; the two heads of a pair run concurrently on
    disjoint PE row groups via tile_position (contraction dim is only 64)
  - exp on ScalarE straight out of PSUM with the 1/sqrt(64) scale folded in;
    no max-subtraction (scores are ~N(0,1) by construction); probs for a
    whole block are buffered in SBUF so the PV matmuls run as a dense PE
    burst - the bursts keep the HAM clock gate at full rate while the
    ACT-bound exp stream is the actual pacing constraint
  - v / second-pair q,k / output projection are emitted as PE filler inside
    the ACT-bound phases; all shared buffers are split into per-block tiles
    because Tile tracks dependencies per tile, not per slice
  - output projection is computed transposed (yT = w_oT_chunks @ attnT);
    the host sums the four partial yT per batch and transposes back.

The attention_mask input is all zeros per the problem spec; a nonzero mask
falls back to an exact host computation.
"""
import sys

sys.path.insert(0, "/opt/trn_rl_repo")

import numpy as np

import concourse.bacc as bacc
import concourse.mybir as mybir
import concourse.tile as tile
from concourse.bass_utils import run_bass_kernel_spmd

B, S, H = 2, 2048, 1024
NH, HD = 16, 64
SCALE = float(np.sqrt(HD))
F32 = mybir.dt.float32
BF16 = mybir.dt.bfloat16
AF = mybir.ActivationFunctionType

_NC_CACHE = None


class Ctx:
    pass


def _qk_sb(nc, c, j, pair, sb):
    """One 512-col chunk of the q (j=0) or k (j=1) projection for a pair."""
    dest = (c.qT if j == 0 else c.kT)[pair][sb]
    col = j * 256 + pair * 128
    ps = c.ps_mm.tile([128, 512], F32, tag="qk", name=f"qk{j}{pair}{sb}")
    for hc in range(8):
        nc.tensor.matmul(ps[:], c.wT[:, hc, col:col + 128],
                         c.hidT[:, hc, sb * 512:(sb + 1) * 512],
                         start=(hc == 0), stop=(hc == 7))
    nc.vector.tensor_copy(dest[:], ps[:])


def _v_chunk(nc, c, sc):
    """One 128-row chunk of the v projection (all 4 heads)."""
    ps = c.ps_mm.tile([128, 256], F32, tag="qk", name=f"vps{sc}")
    for hc in range(8):
        nc.tensor.matmul(ps[:], c.hidT[:, hc, sc * 128:(sc + 1) * 128],
                         c.wT[:, hc, 512:768],
                         start=(hc == 0), stop=(hc == 7))
    vt = c.vA[sc // 4]
    for h in range(4):
        nc.vector.tensor_copy(vt[:, sc % 4, h * 128:h * 128 + 64],
                              ps[:, h * 64:(h + 1) * 64])
    for h in range(4):
        nc.vector.tensor_copy(vt[:, sc % 4, h * 128 + 64:h * 128 + 128],
                              c.ones[:])


def _attn_scores(nc, c, pair, qb, prM, filler=()):
    """A-phase: scores + exp for one block; probs land in SBUF (prM).
    The PE is mostly idle here (ACT-bound), so extra work rides along."""
    filler = list(filler)
    for kc in range(16):
        kt = c.kT[pair][kc // 4]
        kcs = slice((kc % 4) * 128, (kc % 4 + 1) * 128)
        qt = c.qT[pair][qb]
        sW = c.ps_s.tile([128, 1024], F32, tag="sW", name=f"sW{pair}{qb}{kc}")
        nc.tensor.matmul(sW[:, 0:512], kt[0:64, kcs], qt[0:64, :],
                         start=True, stop=True, tile_position=(0, 0))
        nc.tensor.matmul(sW[:, 512:1024], kt[64:128, kcs], qt[64:128, :],
                         start=True, stop=True, tile_position=(64, 0))
        nc.scalar.activation(prM[:, kc, :], sW[:], AF.Exp, scale=1.0 / SCALE)
        if kc % 2 == 1 and filler:
            filler.pop(0)()
    for f in filler:
        f()


def _attn_pv(nc, c, pair, qb, prM, filler=()):
    """B-phase: dense PV burst + normalization for one block."""
    filler = list(filler)
    pv0 = c.ps_pv.tile([128, 512], F32, tag="pv0", name=f"pv0_{pair}{qb}")
    pv1 = c.ps_pv.tile([128, 512], F32, tag="pv1", name=f"pv1_{pair}{qb}")
    c0 = (2 * pair) * 128
    c1 = (2 * pair + 1) * 128
    for kc in range(16):
        vt = c.vA[kc // 4][:, kc % 4, :]
        nc.tensor.matmul(pv0[:], vt[:, c0:c0 + 128], prM[:, kc, 0:512],
                         start=(kc == 0), stop=(kc == 15))
        nc.tensor.matmul(pv1[:], vt[:, c1:c1 + 128], prM[:, kc, 512:1024],
                         start=(kc == 0), stop=(kc == 15))
        if kc % 2 == 1 and filler:
            filler.pop(0)()
    # drain the PV psums into SBUF immediately so the next block's PV
    # matmuls are not gated on the (slow) reciprocal
    pvc = c.recips.tile([128, 512], F32, tag="pvc", name=f"pvc{pair}{qb}")
    den = c.recips.tile([128, 512], F32, tag="den", name=f"den{pair}{qb}")
    nc.vector.tensor_copy(pvc[0:64, :], pv0[0:64, :])
    nc.vector.tensor_copy(pvc[64:128, :], pv1[0:64, :])
    nc.vector.tensor_copy(den[0:64, :], pv0[64:128, :])
    nc.vector.tensor_copy(den[64:128, :], pv1[64:128, :])
    rc = c.recips.tile([128, 512], F32, tag="rc", name=f"rc{pair}{qb}")
    nc.vector.reciprocal_approx_fast(rc[:], den[:])
    nc.vector.tensor_mul(c.attnT[pair][:, qb * 512:(qb + 1) * 512],
                         pvc[:], rc[:])
    for f in filler:
        f()


def _attn_fused(nc, c, pair, qb, prM, filler=()):
    """Last block: scores/exp/PV interleaved per k-chunk so the PV tail after
    the final exp is only one chunk deep (no burst to hide it under)."""
    filler = list(filler)
    pv0 = c.ps_pv.tile([128, 512], F32, tag="pv0", name=f"fpv0_{pair}{qb}")
    pv1 = c.ps_pv.tile([128, 512], F32, tag="pv1", name=f"fpv1_{pair}{qb}")
    c0 = (2 * pair) * 128
    c1 = (2 * pair + 1) * 128
    qt = c.qT[pair][qb]
    for kc in range(16):
        kt = c.kT[pair][kc // 4]
        kcs = slice((kc % 4) * 128, (kc % 4 + 1) * 128)
        sW = c.ps_s.tile([128, 1024], F32, tag="sW", name=f"fsW{kc}")
        nc.tensor.matmul(sW[:, 0:512], kt[0:64, kcs], qt[0:64, :],
                         start=True, stop=True, tile_position=(0, 0))
        nc.tensor.matmul(sW[:, 512:1024], kt[64:128, kcs], qt[64:128, :],
                         start=True, stop=True, tile_position=(64, 0))
        nc.scalar.activation(prM[:, kc, :], sW[:], AF.Exp, scale=1.0 / SCALE)
        vt = c.vA[kc // 4][:, kc % 4, :]
        nc.tensor.matmul(pv0[:], vt[:, c0:c0 + 128], prM[:, kc, 0:512],
                         start=(kc == 0), stop=(kc == 15))
        nc.tensor.matmul(pv1[:], vt[:, c1:c1 + 128], prM[:, kc, 512:1024],
                         start=(kc == 0), stop=(kc == 15))
        if kc % 2 == 1 and filler:
            filler.pop(0)()
    pvc = c.recips.tile([128, 512], F32, tag="pvc", name=f"fpvc{pair}{qb}")
    den = c.recips.tile([128, 512], F32, tag="den", name=f"fden{pair}{qb}")
    nc.vector.tensor_copy(den[0:64, :], pv0[64:128, :])
    nc.vector.tensor_copy(den[64:128, :], pv1[64:128, :])
    rc = c.recips.tile([128, 512], F32, tag="rc", name=f"frc{pair}{qb}")
    nc.vector.reciprocal_approx_fast(rc[:], den[:])
    nc.vector.tensor_copy(pvc[0:64, :], pv0[0:64, :])
    nc.vector.tensor_copy(pvc[64:128, :], pv1[0:64, :])
    nc.vector.tensor_mul(c.attnT[pair][:, qb * 512:(qb + 1) * 512],
                         pvc[:], rc[:])
    for f in filler:
        f()


def _attn_fused(nc, c, pair, qb, prM, filler=()):
    """Last block: scores/exp/PV interleaved per k-chunk so the PV tail after
    the final exp is only one chunk deep (no burst to hide it under)."""
    filler = list(filler)
    pv0 = c.ps_pv.tile([128, 512], F32, tag="pv0", name=f"fpv0_{pair}{qb}")
    pv1 = c.ps_pv.tile([128, 512], F32, tag="pv1", name=f"fpv1_{pair}{qb}")
    c0 = (2 * pair) * 128
    c1 = (2 * pair + 1) * 128
    qt = c.qT[pair][qb]
    for kc in range(16):
        kt = c.kT[pair][kc // 4]
        kcs = slice((kc % 4) * 128, (kc % 4 + 1) * 128)
        sW = c.ps_s.tile([128, 1024], F32, tag="sW", name=f"fsW{kc}")
        nc.tensor.matmul(sW[:, 0:512], kt[0:64, kcs], qt[0:64, :],
                         start=True, stop=True, tile_position=(0, 0))
        nc.tensor.matmul(sW[:, 512:1024], kt[64:128, kcs], qt[64:128, :],
                         start=True, stop=True, tile_position=(64, 0))
        nc.scalar.activation(prM[:, kc, :], sW[:], AF.Exp, scale=1.0 / SCALE)
        vt = c.vA[kc // 4][:, kc % 4, :]
        nc.tensor.matmul(pv0[:], vt[:, c0:c0 + 128], prM[:, kc, 0:512],
                         start=(kc == 0), stop=(kc == 15))
        nc.tensor.matmul(pv1[:], vt[:, c1:c1 + 128], prM[:, kc, 512:1024],
                         start=(kc == 0), stop=(kc == 15))
        if kc % 2 == 1 and filler:
            filler.pop(0)()
    pvc = c.recips.tile([128, 512], F32, tag="pvc", name=f"fpvc{pair}{qb}")
    den = c.recips.tile([128, 512], F32, tag="den", name=f"fden{pair}{qb}")
    nc.vector.tensor_copy(den[0:64, :], pv0[64:128, :])
    nc.vector.tensor_copy(den[64:128, :], pv1[64:128, :])
    rc = c.recips.tile([128, 512], F32, tag="rc", name=f"frc{pair}{qb}")
    nc.vector.reciprocal_approx_fast(rc[:], den[:])
    nc.vector.tensor_copy(pvc[0:64, :], pv0[0:64, :])
    nc.vector.tensor_copy(pvc[64:128, :], pv1[0:64, :])
    nc.vector.tensor_mul(c.attnT[pair][:, qb * 512:(qb + 1) * 512],
                         pvc[:], rc[:])
    for f in filler:
        f()


def _oproj_hoc(nc, c, qb, hoc, y_qb, yT_dst, dcs=(0, 1)):
    psy = c.ps_mm.tile([128, 512], F32, tag="qk", name=f"psy{qb}{hoc}{dcs[0]}")
    for dc in dcs:
        nc.tensor.matmul(psy[:], c.woT[:, dc, hoc * 128:(hoc + 1) * 128],
                         c.attnT[dc][:, qb * 512:(qb + 1) * 512],
                         start=(dc == dcs[0]), stop=(dc == dcs[-1]))
    nc.vector.tensor_copy(y_qb[:, hoc, :], psy[:])
    if hoc == 7:
        nc.sync.dma_start(yT_dst[:, :, qb * 512:(qb + 1) * 512], y_qb[:])


def _oproj_fillers(nc, c, qb, yT_dst, dcs=(0, 1)):
    """One qb-column of the output projection. When dcs is a single half,
    the partial product goes to its own DRAM output (summed on the host
    with the other cores' partials), so no y tile outlives its block."""
    y_qb = c.ysb.tile([128, 8, 512], F32, tag="y", name=f"yqb{qb}d{dcs[0]}")
    return [(lambda hoc=hoc: _oproj_hoc(nc, c, qb, hoc, y_qb, yT_dst, dcs))
            for hoc in range(8)]


def _emit(tc, yT, yT2, hid, wqkv, wo):
    nc = tc.nc
    c = Ctx()
    yT_p = yT.rearrange("(t p) c -> p t c", p=128)        # [128, 8, 2048]
    yT2_p = yT2.rearrange("(t p) c -> p t c", p=128)

    with tc.tile_pool(name="persist", bufs=1) as persist, \
         tc.tile_pool(name="ps_mm", bufs=2, space="PSUM") as ps_mm:
        c.ps_mm = ps_mm
        c.wT = persist.tile([128, 8, 768], BF16)    # w_qkv_slice.T (h-major)
        c.woT = persist.tile([128, 2, 1024], BF16)  # w_o_slice.T   (d-major)
        # per-block tiles: Tile tracks deps per tile, so fine-grained tiles
        # let independent work interleave without false serialization
        c.qT = [[persist.tile([128, 512], BF16, name=f"qT{p}{sb}")
                 for sb in range(4)] for p in range(2)]
        c.kT = [[persist.tile([128, 512], BF16, name=f"kT{p}{sb}")
                 for sb in range(4)] for p in range(2)]
        c.vA = [persist.tile([128, 4, 512], BF16, name=f"vA{g}")
                for g in range(4)]
        c.attnT = [persist.tile([128, 2048], BF16, name=f"attnT{p}")
                   for p in range(2)]
        c.ones = persist.tile([128, 64], BF16)
        nc.vector.memset(c.ones[:], 1.0)

        # weight loads split by q/k/v column group and hidden by 512-column
        # s-blocks, ordered so the first k/q projection chunks (and with them
        # attention block 0) start as early as possible
        # dependency-free warm-up matmuls: keep the PE busy during the
        # initial DMA wait so the HAM clock gate reaches full rate before the
        # first real projection chain
        warm = persist.tile([128, 512], BF16, name="warm")
        nc.vector.memset(warm[:], 1.0)
        wps = ps_mm.tile([128, 512], F32, tag="qk", name="warmps")
        for i in range(32):
            nc.tensor.matmul(wps[:], warm[:, 0:128], warm[:],
                             start=(i == 0), stop=(i == 31))

        wT_pt = wqkv.rearrange("(t p) c -> p t c", p=128)   # [128, 8, 768]
        c.hidT = persist.tile([128, 8, 2048], BF16)
        hidT_pt = hid.rearrange("(t p) c -> p t c", p=128)  # [128, 8, 2048]
        nc.gpsimd.dma_start(c.wT[:, :, 256:512], wT_pt[:, :, 256:512])  # wk
        for sb in range(4):
            ssl = slice(sb * 512, (sb + 1) * 512)
            nc.gpsimd.dma_start(c.hidT[:, :, ssl], hidT_pt[:, :, ssl])
            _qk_sb(nc, c, 1, 0, sb)   # k0 chunk for this s-block
            if sb == 0:
                nc.gpsimd.dma_start(c.wT[:, :, 0:256], wT_pt[:, :, 0:256])  # wq
            _qk_sb(nc, c, 0, 0, sb)   # q0 chunk for this s-block
        nc.gpsimd.dma_start(c.wT[:, :, 512:768], wT_pt[:, :, 512:768])      # wv
        woT_pt = wo.rearrange("(t p) c -> t p c", p=128)    # [2, 128, 1024]
        for dc in range(2):
            nc.gpsimd.dma_start(c.woT[:, dc, :], woT_pt[dc])

        with tc.tile_pool(name="probs", bufs=2) as probs, \
             tc.tile_pool(name="recips", bufs=2) as recips, \
             tc.tile_pool(name="ysb", bufs=2) as ysb, \
             tc.tile_pool(name="ps_s", bufs=2, space="PSUM") as ps_s, \
             tc.tile_pool(name="ps_pv", bufs=1, space="PSUM") as ps_pv:
            c.probs, c.recips, c.ysb = probs, recips, ysb
            c.ps_s, c.ps_pv = ps_s, ps_pv

            blocks = [(p, qb) for p in range(2) for qb in range(4)]
            # A-phase fillers: v chunks during A0/A1, pair-1 q/k during A2/A3
            v_fill = [(lambda sc=sc: _v_chunk(nc, c, sc)) for sc in range(16)]
            qk_fill = [(lambda j=j, sb=sb: _qk_sb(nc, c, j, 1, sb))
                       for j in (1, 0) for sb in range(4)]
            a_fill = {0: v_fill[0:8], 1: v_fill[8:16],
                      2: qk_fill[0:4], 3: qk_fill[4:8]}

            prMs = {}
            prMs[0] = probs.tile([128, 16, 1024], BF16, tag="prM", name="prM0")
            _attn_scores(nc, c, *blocks[0], prMs[0], a_fill.get(0, ()))
            for i in range(7):
                if i + 1 < 7:
                    prMs[i + 1] = probs.tile([128, 16, 1024], BF16, tag="prM",
                                             name=f"prM{i + 1}")
                    fillA = a_fill.get(i + 1, ())
                    if i + 1 == 5:
                        # pair-0 half of oproj(2): only needs pair-0's attnT
                        fillA = _oproj_fillers(nc, c, 2, yT2_p, dcs=(0,))
                    _attn_scores(nc, c, *blocks[i + 1], prMs[i + 1], fillA)
                p, qb = blocks[i]
                # oproj(qb-1) becomes available once pair-1's block qb-1 is
                # done; oproj(3)'s pair-0 half only needs pair-0's attnT
                if p == 1 and qb in (1, 2):
                    fill = _oproj_fillers(nc, c, qb - 1, yT_p)
                elif p == 1 and qb == 0:
                    fill = _oproj_fillers(nc, c, 3, yT2_p, dcs=(0,))
                else:
                    fill = ()
                _attn_pv(nc, c, p, qb, prMs.pop(i), fill)
            # last block fused (short tail); its fillers finish oproj(2)
            prM7 = probs.tile([128, 16, 1024], BF16, tag="prM", name="prM7")
            _attn_fused(nc, c, *blocks[7], prM7, ())
            for f in _oproj_fillers(nc, c, 2, yT_p, dcs=(1,)):
                f()
            for f in _oproj_fillers(nc, c, 3, yT_p, dcs=(1,)):
                f()


def build_nc():
    global _NC_CACHE
    if _NC_CACHE is not None:
        return _NC_CACHE
    nc = bacc.Bacc("TRN2", target_bir_lowering=False, debug=False, num_devices=8)
    hid = nc.dram_tensor("hid", [H, S], F32, kind="ExternalInput").ap()
    wqkv = nc.dram_tensor("wqkv", [H, 768], F32, kind="ExternalInput").ap()
    wo = nc.dram_tensor("wo", [256, H], F32, kind="ExternalInput").ap()
    yT = nc.dram_tensor("yT", [H, S], F32, kind="ExternalOutput").ap()
    yT2 = nc.dram_tensor("yT2", [H, S], F32, kind="ExternalOutput").ap()
    with tile.TileContext(nc) as tc:
        _emit(tc, yT, yT2, hid, wqkv, wo)
    nc.compile()
    _NC_CACHE = nc
    return nc


def _host_reference(hidden_states, attention_mask, w_qkv, w_o):
    """Exact numpy fallback (used only if the mask is nonzero)."""
    h = hidden_states.astype(np.float32)
    qkv = h @ w_qkv.T
    qkv = qkv.reshape(B, S, 3, NH, HD).transpose(2, 0, 3, 1, 4)
    q, k, v = qkv[0], qkv[1], qkv[2]
    s = np.einsum("bhqd,bhkd->bhqk", q, k) / SCALE + attention_mask[:, None]
    s -= s.max(-1, keepdims=True)
    p = np.exp(s)
    p /= p.sum(-1, keepdims=True)
    a = np.einsum("bhqk,bhkd->bhqd", p, v)
    a = a.transpose(0, 2, 1, 3).reshape(B, S, H)
    return (a @ w_o.T).astype(np.float32)


def _install_ntff_hook():
    """Provide antenv.axon_hooks (missing on this image) so trace=True works."""
    import types

    try:
        import antenv.axon_hooks  # noqa: F401
        return
    except ImportError:
        pass
    hook = None
    try:
        sys.path.insert(0, "/root/.axon_site")
        from trn_agent_boot.trn_boot import _ntff_profile_via_ctypes
        hook = _ntff_profile_via_ctypes("/opt/axon/libaxon_pjrt.so")
    except Exception:
        hook = None
    mod = types.ModuleType("antenv.axon_hooks")
    state = {"hook": hook}
    mod.get_axon_ntff_profile_hook = lambda: state["hook"]
    mod.set_axon_ntff_profile_hook = lambda h: state.__setitem__("hook", h)
    sys.modules["antenv.axon_hooks"] = mod
    import antenv
    antenv.axon_hooks = mod


def kernel(hidden_states, attention_mask, w_qkv, w_o, _trace=False):
    if _trace:
        _install_ntff_hook()
    hidden_states = np.asarray(hidden_states, dtype=np.float32)
    attention_mask = np.asarray(attention_mask, dtype=np.float32)
    w_qkv = np.asarray(w_qkv, dtype=np.float32)
    w_o = np.asarray(w_o, dtype=np.float32)
    if attention_mask.size and np.abs(attention_mask).max() != 0.0:
        return _host_reference(hidden_states, attention_mask, w_qkv, w_o)

    in_maps = []
    for cid in range(8):
        b, hp = divmod(cid, 4)
        r = slice(hp * 256, hp * 256 + 256)
        wq_slice = np.concatenate([w_qkv[0:1024][r], w_qkv[1024:2048][r],
                                   w_qkv[2048:3072][r]], axis=0)
        in_maps.append({
            "hid": np.ascontiguousarray(hidden_states[b].T),
            "wqkv": np.ascontiguousarray(wq_slice.T),
            "wo": np.ascontiguousarray(w_o[:, r].T),
        })
    nc = build_nc()
    res = run_bass_kernel_spmd(nc, in_maps, core_ids=list(range(8)), trace=_trace)
    outs = [r["yT"] + r["yT2"] for r in res.results]
    y = np.empty((B, S, H), dtype=np.float32)
    for b in range(B):
        acc = outs[4 * b] + outs[4 * b + 1] + outs[4 * b + 2] + outs[4 * b + 3]
        y[b] = acc.T
    if _trace:
        kernel._last_results = res
    return y


# revision 32
# speedup vs baseline: 1.0017x; 1.0017x over previous
"""Multi-head attention (B=2, S=2048, H=1024, 16 heads x 64) on 8 Trainium2 cores.

Sharding: tensor-parallel over heads x data-parallel over batch.
Core c handles batch b = c//4 and heads [4*(c%4), 4*(c%4)+4).

Per-core kernel (bf16 matmul operands, fp32 PSUM accumulation):
  - inputs arrive host-transposed (contraction dim outermost) as part of the
    sharding step; SWDGE cast-loads move them straight into bf16 SBUF
  - QKV projection produces qT/kT ([headdim, S], head pairs stacked on
    partitions) and V in natural [S, headdim] layout augmented with ones
    columns; the ones rows of the PV output give the softmax denominators
    replicated across 64 partitions, so normalization needs no broadcast
  - scores are computed transposed (sT = kT_chunk.T @ qT block) so the
    softmax k-reduction lands on the partition axis and probs come out in
    the [k, q] layout PV needs; the two heads of a pair run concurrently on
    disjoint PE row groups via tile_position (contraction dim is only 64)
  - exp on ScalarE straight out of PSUM with the 1/sqrt(64) scale folded in;
    no max-subtraction (scores are ~N(0,1) by construction); probs for a
    whole block are buffered in SBUF so the PV matmuls run as a dense PE
    burst - the bursts keep the HAM clock gate at full rate while the
    ACT-bound exp stream is the actual pacing constraint
  - v / second-pair q,k / output projection are emitted as PE filler inside
    the ACT-bound phases; all shared buffers are split into per-block tiles
    because Tile tracks dependencies per tile, not per slice
  - output projection is computed transposed (yT = w_oT_chunks @ attnT);
    the host sums the four partial yT per batch and transposes back.

The attention_mask input is all zeros per the problem spec; a nonzero mask
falls back to an exact host computation.
"""
import sys

sys.path.insert(0, "/opt/trn_rl_repo")

import numpy as np

import concourse.bacc as bacc
import concourse.mybir as mybir
import concourse.tile as tile
from concourse.bass_utils import run_bass_kernel_spmd

B, S, H = 2, 2048, 1024
NH, HD = 16, 64
SCALE = float(np.sqrt(HD))
F32 = mybir.dt.float32
BF16 = mybir.dt.bfloat16
AF = mybir.ActivationFunctionType

_NC_CACHE = None


class Ctx:
    pass


def _qk_sb(nc, c, j, pair, sb):
    """One 512-col chunk of the q (j=0) or k (j=1) projection for a pair."""
    dest = (c.qT if j == 0 else c.kT)[pair][sb]
    col = j * 256 + pair * 128
    ps = c.ps_mm.tile([128, 512], F32, tag="qk", name=f"qk{j}{pair}{sb}")
    for hc in range(8):
        nc.tensor.matmul(ps[:], c.wT[:, hc, col:col + 128],
                         c.hidT[:, hc, sb * 512:(sb + 1) * 512],
                         start=(hc == 0), stop=(hc == 7))
    nc.vector.tensor_copy(dest[:], ps[:])


def _v_chunk(nc, c, sc):
    """One 128-row chunk of the v projection (all 4 heads)."""
    ps = c.ps_mm.tile([128, 256], F32, tag="qk", name=f"vps{sc}")
    for hc in range(8):
        nc.tensor.matmul(ps[:], c.hidT[:, hc, sc * 128:(sc + 1) * 128],
                         c.wT[:, hc, 512:768],
                         start=(hc == 0), stop=(hc == 7))
    vt = c.vA[sc // 4]
    for h in range(4):
        nc.vector.tensor_copy(vt[:, sc % 4, h * 128:h * 128 + 64],
                              ps[:, h * 64:(h + 1) * 64])
    for h in range(4):
        nc.vector.tensor_copy(vt[:, sc % 4, h * 128 + 64:h * 128 + 128],
                              c.ones[:])


def _attn_scores(nc, c, pair, qb, prM, filler=()):
    """A-phase: scores + exp for one block; probs land in SBUF (prM).
    The PE is mostly idle here (ACT-bound), so extra work rides along."""
    filler = list(filler)
    for kc in range(16):
        kt = c.kT[pair][kc // 4]
        kcs = slice((kc % 4) * 128, (kc % 4 + 1) * 128)
        qt = c.qT[pair][qb]
        sW = c.ps_s.tile([128, 1024], F32, tag="sW", name=f"sW{pair}{qb}{kc}")
        nc.tensor.matmul(sW[:, 0:512], kt[0:64, kcs], qt[0:64, :],
                         start=True, stop=True, tile_position=(0, 0))
        nc.tensor.matmul(sW[:, 512:1024], kt[64:128, kcs], qt[64:128, :],
                         start=True, stop=True, tile_position=(64, 0))
        nc.scalar.activation(prM[:, kc, :], sW[:], AF.Exp, scale=1.0 / SCALE)
        if kc % 2 == 1 and filler:
            filler.pop(0)()
    for f in filler:
        f()


def _attn_pv(nc, c, pair, qb, prM, filler=()):
    """B-phase: dense PV burst + normalization for one block."""
    filler = list(filler)
    pv0 = c.ps_pv.tile([128, 512], F32, tag="pv0", name=f"pv0_{pair}{qb}")
    pv1 = c.ps_pv.tile([128, 512], F32, tag="pv1", name=f"pv1_{pair}{qb}")
    c0 = (2 * pair) * 128
    c1 = (2 * pair + 1) * 128
    for kc in range(16):
        vt = c.vA[kc // 4][:, kc % 4, :]
        nc.tensor.matmul(pv0[:], vt[:, c0:c0 + 128], prM[:, kc, 0:512],
                         start=(kc == 0), stop=(kc == 15))
        nc.tensor.matmul(pv1[:], vt[:, c1:c1 + 128], prM[:, kc, 512:1024],
                         start=(kc == 0), stop=(kc == 15))
        if kc % 2 == 1 and filler:
            filler.pop(0)()
    # drain the PV psums into SBUF immediately so the next block's PV
    # matmuls are not gated on the (slow) reciprocal
    pvc = c.recips.tile([128, 512], F32, tag="pvc", name=f"pvc{pair}{qb}")
    den = c.recips.tile([128, 512], F32, tag="den", name=f"den{pair}{qb}")
    nc.vector.tensor_copy(pvc[0:64, :], pv0[0:64, :])
    nc.vector.tensor_copy(pvc[64:128, :], pv1[0:64, :])
    nc.vector.tensor_copy(den[0:64, :], pv0[64:128, :])
    nc.vector.tensor_copy(den[64:128, :], pv1[64:128, :])
    rc = c.recips.tile([128, 512], F32, tag="rc", name=f"rc{pair}{qb}")
    nc.vector.reciprocal_approx_fast(rc[:], den[:])
    nc.vector.tensor_mul(c.attnT[pair][:, qb * 512:(qb + 1) * 512],
                         pvc[:], rc[:])
    for f in filler:
        f()


def _attn_fused(nc, c, pair, qb, prM, filler=()):
    """Last block: scores/exp/PV interleaved per k-chunk so the PV tail after
    the final exp is only one chunk deep (no burst to hide it under)."""
    filler = list(filler)
    pv0 = c.ps_pv.tile([128, 512], F32, tag="pv0", name=f"fpv0_{pair}{qb}")
    pv1 = c.ps_pv.tile([128, 512], F32, tag="pv1", name=f"fpv1_{pair}{qb}")
    c0 = (2 * pair) * 128
    c1 = (2 * pair + 1) * 128
    qt = c.qT[pair][qb]
    for kc in range(16):
        kt = c.kT[pair][kc // 4]
        kcs = slice((kc % 4) * 128, (kc % 4 + 1) * 128)
        sW = c.ps_s.tile([128, 1024], F32, tag="sW", name=f"fsW{kc}")
        nc.tensor.matmul(sW[:, 0:512], kt[0:64, kcs], qt[0:64, :],
                         start=True, stop=True, tile_position=(0, 0))
        nc.tensor.matmul(sW[:, 512:1024], kt[64:128, kcs], qt[64:128, :],
                         start=True, stop=True, tile_position=(64, 0))
        nc.scalar.activation(prM[:, kc, :], sW[:], AF.Exp, scale=1.0 / SCALE)
        vt = c.vA[kc // 4][:, kc % 4, :]
        nc.tensor.matmul(pv0[:], vt[:, c0:c0 + 128], prM[:, kc, 0:512],
                         start=(kc == 0), stop=(kc == 15))
        nc.tensor.matmul(pv1[:], vt[:, c1:c1 + 128], prM[:, kc, 512:1024],
                         start=(kc == 0), stop=(kc == 15))
        if kc % 2 == 1 and filler:
            filler.pop(0)()
    pvc = c.recips.tile([128, 512], F32, tag="pvc", name=f"fpvc{pair}{qb}")
    den = c.recips.tile([128, 512], F32, tag="den", name=f"fden{pair}{qb}")
    nc.vector.tensor_copy(den[0:64, :], pv0[64:128, :])
    nc.vector.tensor_copy(den[64:128, :], pv1[64:128, :])
    rc = c.recips.tile([128, 512], F32, tag="rc", name=f"frc{pair}{qb}")
    nc.vector.reciprocal_approx_fast(rc[:], den[:])
    nc.vector.tensor_copy(pvc[0:64, :], pv0[0:64, :])
    nc.vector.tensor_copy(pvc[64:128, :], pv1[0:64, :])
    nc.vector.tensor_mul(c.attnT[pair][:, qb * 512:(qb + 1) * 512],
                         pvc[:], rc[:])
    for f in filler:
        f()


def _attn_fused(nc, c, pair, qb, prM, filler=()):
    """Last block: scores/exp/PV interleaved per k-chunk so the PV tail after
    the final exp is only one chunk deep (no burst to hide it under)."""
    filler = list(filler)
    pv0 = c.ps_pv.tile([128, 512], F32, tag="pv0", name=f"fpv0_{pair}{qb}")
    pv1 = c.ps_pv.tile([128, 512], F32, tag="pv1", name=f"fpv1_{pair}{qb}")
    c0 = (2 * pair) * 128
    c1 = (2 * pair + 1) * 128
    qt = c.qT[pair][qb]
    for kc in range(16):
        kt = c.kT[pair][kc // 4]
        kcs = slice((kc % 4) * 128, (kc % 4 + 1) * 128)
        sW = c.ps_s.tile([128, 1024], F32, tag="sW", name=f"fsW{kc}")
        nc.tensor.matmul(sW[:, 0:512], kt[0:64, kcs], qt[0:64, :],
                         start=True, stop=True, tile_position=(0, 0))
        nc.tensor.matmul(sW[:, 512:1024], kt[64:128, kcs], qt[64:128, :],
                         start=True, stop=True, tile_position=(64, 0))
        nc.scalar.activation(prM[:, kc, :], sW[:], AF.Exp, scale=1.0 / SCALE)
        vt = c.vA[kc // 4][:, kc % 4, :]
        nc.tensor.matmul(pv0[:], vt[:, c0:c0 + 128], prM[:, kc, 0:512],
                         start=(kc == 0), stop=(kc == 15))
        nc.tensor.matmul(pv1[:], vt[:, c1:c1 + 128], prM[:, kc, 512:1024],
                         start=(kc == 0), stop=(kc == 15))
        if kc % 2 == 1 and filler:
            filler.pop(0)()
    pvc = c.recips.tile([128, 512], F32, tag="pvc", name=f"fpvc{pair}{qb}")
    den = c.recips.tile([128, 512], F32, tag="den", name=f"fden{pair}{qb}")
    nc.vector.tensor_copy(den[0:64, :], pv0[64:128, :])
    nc.vector.tensor_copy(den[64:128, :], pv1[64:128, :])
    rc = c.recips.tile([128, 512], F32, tag="rc", name=f"frc{pair}{qb}")
    nc.vector.reciprocal_approx_fast(rc[:], den[:])
    nc.vector.tensor_copy(pvc[0:64, :], pv0[0:64, :])
    nc.vector.tensor_copy(pvc[64:128, :], pv1[0:64, :])
    nc.vector.tensor_mul(c.attnT[pair][:, qb * 512:(qb + 1) * 512],
                         pvc[:], rc[:])
    for f in filler:
        f()


def _oproj_hoc(nc, c, qb, hoc, y_qb, yT_dst, dcs=(0, 1)):
    psy = c.ps_mm.tile([128, 512], F32, tag="qk", name=f"psy{qb}{hoc}{dcs[0]}")
    for dc in dcs:
        nc.tensor.matmul(psy[:], c.woT[:, dc, hoc * 128:(hoc + 1) * 128],
                         c.attnT[dc][:, qb * 512:(qb + 1) * 512],
                         start=(dc == dcs[0]), stop=(dc == dcs[-1]))
    nc.vector.tensor_copy(y_qb[:, hoc, :], psy[:])
    if hoc == 7:
        nc.sync.dma_start(yT_dst[:, :, qb * 512:(qb + 1) * 512], y_qb[:])


def _oproj_fillers(nc, c, qb, yT_dst, dcs=(0, 1)):
    """One qb-column of the output projection. When dcs is a single half,
    the partial product goes to its own DRAM output (summed on the host
    with the other cores' partials), so no y tile outlives its block."""
    y_qb = c.ysb.tile([128, 8, 512], F32, tag="y", name=f"yqb{qb}d{dcs[0]}")
    return [(lambda hoc=hoc: _oproj_hoc(nc, c, qb, hoc, y_qb, yT_dst, dcs))
            for hoc in range(8)]


def _emit(tc, yT, yT2, hid, wqkv, wo):
    nc = tc.nc
    c = Ctx()
    yT_p = yT.rearrange("(t p) c -> p t c", p=128)        # [128, 8, 2048]
    yT2_p = yT2.rearrange("(t p) c -> p t c", p=128)

    with tc.tile_pool(name="persist", bufs=1) as persist, \
         tc.tile_pool(name="ps_mm", bufs=2, space="PSUM") as ps_mm:
        c.ps_mm = ps_mm
        c.wT = persist.tile([128, 8, 768], BF16)    # w_qkv_slice.T (h-major)
        c.woT = persist.tile([128, 2, 1024], BF16)  # w_o_slice.T   (d-major)
        # per-block tiles: Tile tracks deps per tile, so fine-grained tiles
        # let independent work interleave without false serialization
        c.qT = [[persist.tile([128, 512], BF16, name=f"qT{p}{sb}")
                 for sb in range(4)] for p in range(2)]
        c.kT = [[persist.tile([128, 512], BF16, name=f"kT{p}{sb}")
                 for sb in range(4)] for p in range(2)]
        c.vA = [persist.tile([128, 4, 512], BF16, name=f"vA{g}")
                for g in range(4)]
        c.attnT = [persist.tile([128, 2048], BF16, name=f"attnT{p}")
                   for p in range(2)]
        c.ones = persist.tile([128, 64], BF16)
        nc.vector.memset(c.ones[:], 1.0)

        # weight loads split by q/k/v column group and hidden by 512-column
        # s-blocks, ordered so the first k/q projection chunks (and with them
        # attention block 0) start as early as possible
        # dependency-free warm-up matmuls: keep the PE busy during the
        # initial DMA wait so the HAM clock gate reaches full rate before the
        # first real projection chain
        warm = persist.tile([128, 512], BF16, name="warm")
        nc.vector.memset(warm[:], 1.0)
        wps = ps_mm.tile([128, 512], F32, tag="qk", name="warmps")
        for i in range(32):
            nc.tensor.matmul(wps[:], warm[:, 0:128], warm[:],
                             start=(i == 0), stop=(i == 31))

        wT_pt = wqkv.rearrange("(t p) c -> p t c", p=128)   # [128, 8, 768]
        c.hidT = persist.tile([128, 8, 2048], BF16)
        hidT_pt = hid.rearrange("(t p) c -> p t c", p=128)  # [128, 8, 2048]
        nc.gpsimd.dma_start(c.wT[:, :, 256:512], wT_pt[:, :, 256:512])  # wk
        for sb in range(4):
            ssl = slice(sb * 512, (sb + 1) * 512)
            nc.gpsimd.dma_start(c.hidT[:, :, ssl], hidT_pt[:, :, ssl])
            _qk_sb(nc, c, 1, 0, sb)   # k0 chunk for this s-block
            if sb == 0:
                nc.gpsimd.dma_start(c.wT[:, :, 0:256], wT_pt[:, :, 0:256])  # wq
            _qk_sb(nc, c, 0, 0, sb)   # q0 chunk for this s-block
        nc.gpsimd.dma_start(c.wT[:, :, 512:768], wT_pt[:, :, 512:768])      # wv
        woT_pt = wo.rearrange("(t p) c -> t p c", p=128)    # [2, 128, 1024]
        for dc in range(2):
            nc.gpsimd.dma_start(c.woT[:, dc, :], woT_pt[dc])

        with tc.tile_pool(name="probs", bufs=2) as probs, \
             tc.tile_pool(name="recips", bufs=2) as recips, \
             tc.tile_pool(name="ysb", bufs=2) as ysb, \
             tc.tile_pool(name="ps_s", bufs=2, space="PSUM") as ps_s, \
             tc.tile_pool(name="ps_pv", bufs=1, space="PSUM") as ps_pv:
            c.probs, c.recips, c.ysb = probs, recips, ysb
            c.ps_s, c.ps_pv = ps_s, ps_pv

            blocks = [(p, qb) for p in range(2) for qb in range(4)]
            # A-phase fillers: v chunks during A0/A1, pair-1 q/k during A2/A3
            v_fill = [(lambda sc=sc: _v_chunk(nc, c, sc)) for sc in range(16)]
            qk_fill = [(lambda j=j, sb=sb: _qk_sb(nc, c, j, 1, sb))
                       for j in (1, 0) for sb in range(4)]
            a_fill = {0: v_fill[0:8], 1: v_fill[8:16],
                      2: qk_fill[0:4], 3: qk_fill[4:8]}

            prMs = {}
            prMs[0] = probs.tile([128, 16, 1024], BF16, tag="prM", name="prM0")
            _attn_scores(nc, c, *blocks[0], prMs[0], a_fill.get(0, ()))
            for i in range(7):
                if i + 1 < 7:
                    prMs[i + 1] = probs.tile([128, 16, 1024], BF16, tag="prM",
                                             name=f"prM{i + 1}")
                    fillA = a_fill.get(i + 1, ())
                    if i + 1 == 5:
                        # pair-0 half of oproj(2): only needs pair-0's attnT
                        fillA = _oproj_fillers(nc, c, 2, yT2_p, dcs=(0,))
                    _attn_scores(nc, c, *blocks[i + 1], prMs[i + 1], fillA)
                p, qb = blocks[i]
                # oproj(qb-1) becomes available once pair-1's block qb-1 is
                # done; oproj(3)'s pair-0 half only needs pair-0's attnT
                if p == 1 and qb in (1, 2):
                    fill = _oproj_fillers(nc, c, qb - 1, yT_p)
                elif p == 1 and qb == 0:
                    fill = _oproj_fillers(nc, c, 3, yT2_p, dcs=(0,))
                else:
                    fill = ()
                _attn_pv(nc, c, p, qb, prMs.pop(i), fill)
            # last block fused (short tail); its fillers finish oproj(2)
            prM7 = probs.tile([128, 16, 1024], BF16, tag="prM", name="prM7")
            _attn_fused(nc, c, *blocks[7], prM7,
                        _oproj_fillers(nc, c, 2, yT_p, dcs=(1,)))
            for f in _oproj_fillers(nc, c, 3, yT_p, dcs=(1,)):
                f()


def build_nc():
    global _NC_CACHE
    if _NC_CACHE is not None:
        return _NC_CACHE
    nc = bacc.Bacc("TRN2", target_bir_lowering=False, debug=False, num_devices=8)
    hid = nc.dram_tensor("hid", [H, S], F32, kind="ExternalInput").ap()
    wqkv = nc.dram_tensor("wqkv", [H, 768], F32, kind="ExternalInput").ap()
    wo = nc.dram_tensor("wo", [256, H], F32, kind="ExternalInput").ap()
    yT = nc.dram_tensor("yT", [H, S], F32, kind="ExternalOutput").ap()
    yT2 = nc.dram_tensor("yT2", [H, S], F32, kind="ExternalOutput").ap()
    with tile.TileContext(nc) as tc:
        _emit(tc, yT, yT2, hid, wqkv, wo)
    nc.compile()
    _NC_CACHE = nc
    return nc


def _host_reference(hidden_states, attention_mask, w_qkv, w_o):
    """Exact numpy fallback (used only if the mask is nonzero)."""
    h = hidden_states.astype(np.float32)
    qkv = h @ w_qkv.T
    qkv = qkv.reshape(B, S, 3, NH, HD).transpose(2, 0, 3, 1, 4)
    q, k, v = qkv[0], qkv[1], qkv[2]
    s = np.einsum("bhqd,bhkd->bhqk", q, k) / SCALE + attention_mask[:, None]
    s -= s.max(-1, keepdims=True)
    p = np.exp(s)
    p /= p.sum(-1, keepdims=True)
    a = np.einsum("bhqk,bhkd->bhqd", p, v)
    a = a.transpose(0, 2, 1, 3).reshape(B, S, H)
    return (a @ w_o.T).astype(np.float32)


def _install_ntff_hook():
    """Provide antenv.axon_hooks (missing on this image) so trace=True works."""
    import types

    try:
        import antenv.axon_hooks  # noqa: F401
        return
    except ImportError:
        pass
    hook = None
    try:
        sys.path.insert(0, "/root/.axon_site")
        from trn_agent_boot.trn_boot import _ntff_profile_via_ctypes
        hook = _ntff_profile_via_ctypes("/opt/axon/libaxon_pjrt.so")
    except Exception:
        hook = None
    mod = types.ModuleType("antenv.axon_hooks")
    state = {"hook": hook}
    mod.get_axon_ntff_profile_hook = lambda: state["hook"]
    mod.set_axon_ntff_profile_hook = lambda h: state.__setitem__("hook", h)
    sys.modules["antenv.axon_hooks"] = mod
    import antenv
    antenv.axon_hooks = mod


def kernel(hidden_states, attention_mask, w_qkv, w_o, _trace=False):
    if _trace:
        _install_ntff_hook()
    hidden_states = np.asarray(hidden_states, dtype=np.float32)
    attention_mask = np.asarray(attention_mask, dtype=np.float32)
    w_qkv = np.asarray(w_qkv, dtype=np.float32)
    w_o = np.asarray(w_o, dtype=np.float32)
    if attention_mask.size and np.abs(attention_mask).max() != 0.0:
        return _host_reference(hidden_states, attention_mask, w_qkv, w_o)

    in_maps = []
    for cid in range(8):
        b, hp = divmod(cid, 4)
        r = slice(hp * 256, hp * 256 + 256)
        wq_slice = np.concatenate([w_qkv[0:1024][r], w_qkv[1024:2048][r],
                                   w_qkv[2048:3072][r]], axis=0)
        in_maps.append({
            "hid": np.ascontiguousarray(hidden_states[b].T),
            "wqkv": np.ascontiguousarray(wq_slice.T),
            "wo": np.ascontiguousarray(w_o[:, r].T),
        })
    nc = build_nc()
    res = run_bass_kernel_spmd(nc, in_maps, core_ids=list(range(8)), trace=_trace)
    outs = [r["yT"] + r["yT2"] for r in res.results]
    y = np.empty((B, S, H), dtype=np.float32)
    for b in range(B):
        acc = outs[4 * b] + outs[4 * b + 1] + outs[4 * b + 2] + outs[4 * b + 3]
        y[b] = acc.T
    if _trace:
        kernel._last_results = res
    return y
